# revision 49
# baseline (speedup 1.0000x reference)
"""Bass/Trainium2 kernel for a 2-layer bidirectional QRNN (fo-pooling).

Reference computation (per layer, per direction):
    ZFO = X @ W + b            # [S, B, 3H]
    Z, F, O = split(ZFO); Z = tanh(Z); F = sigmoid(F); O = sigmoid(O)
    c_t = F_t * c_{t-1} + (1 - F_t) * Z_t        (bw direction: reversed time)
    Y_dir = O * C
    Y = concat(Y_fw, Y_bw)     # [S, B, 2H]
Two stacked layers; output is [B, S, 2H].

Sharding: data-parallel over batch. B=16 rows -> 2 rows per NeuronCore x 8.
Each core runs both layers for its 2 rows; no collectives.

Design (635us naive -> 523.9us prior best -> ~518us current; measured
in-stream PE busy ~499us vs a 491.5us pure-streaming floor):
- The kernel is tensor-engine-bound: 2304 matmuls x 512 moving rows at
  1 cyc/row / 2.4GHz = 491.5us of pure PE streaming per core.  fp8 was
  fully explored and is a dead end on THIS silicon, but not for accuracy:
  O-gate-only fp8 PASSES the 2e-2 gate (1.70e-2 HW-measured; all-gate
  fp8 is 7.7e-2, F/O-only 2.7e-2 -- fails).  It dies on throughput: a
  DoubleRow matmul with rhs [128,2,512] measures ~900 cycles (streams
  the full 1024-element free size at ~1/cycle, no 2-rows/cycle win for
  large free dims) plus ~480ns/group of exposed 256-col DR LDWEIGHTS,
  so 2 DR matmuls cost MORE than the 4 fp16 matmuls they replace.  fp32r
  also runs 1 cyc/row for free-dim >= 256, so fp16 is optimal: same PE
  rate, 2-byte LDWEIGHTS fully shadow-loaded, direct DMA of host-pre-cast
  operands.  Whole-pipeline fp16 rel-err is 1.37e-3.
- X ([BC,D,S] fp16, 4MB/core) and the layer-0 output y1 ([BC,2H,S] fp16,
  8MB/core) are SBUF-resident; y1 never round-trips DRAM. HBM traffic drops
  ~103MB -> ~22MB/core.  Total DMA count is ~41 (batched weight/bias/X
  loads; layer-1 outputs accumulate into full-row tiles and ship as one
  [128,2048] DMA per (b, dir-hh), except the last pass's last row which
  keeps per-s-tile writes for a short tail).  Note: teardown cost is
  measured invariant to DMA count -- the batching just simplifies the
  sequencer schedule.
- Gate combine fused: one DVE scalar_tensor_tensor computes g' = (f-1)*z and
  the DVE scan runs state = f*state - g' = f*state + (1-f)*z (fp32 internal
  state), so per [128,512] tile the non-PE work is 3 scalar activations +
  2 DVE ops + 1 gpsimd mul + 1 copy -- all hidden under the 12 (layer0) /
  24 (layer1) matmuls per tile.
- Pass order l0-fw, l0-bw, l1-bw, l1-fw with byte-range hazard tracking:
  measured total in-stream matmul gap is now ~0.3us over 500us.
- The graded exec window opens at the first engine instruction (~5.8us,
  framework-emitted memsets) regardless of kernel structure, and operand
  payloads cannot land before ~12.2us (sequencer boot ~7.2 + ~0.7us
  DIRECT2D issue per DMA + ring FIFO payload drain).  So: 10 free warmup
  matmuls bridge PE-boot to first-payload and flip the HAM clock gate
  (1.2->2.4GHz needs ~3.4us of sustained PE activity) so real work starts
  warm; hh-major weight layout + s-sliced X loads make the first-needed
  bytes land first (w0f-hh0 unlocks 48 matmuls); the first s-tile runs
  h2-OUTER so its first 48 matmuls consume only X[s0:256].
- ALL output DMAs ride the sync ring, and the scalar ring carries EXACTLY
  the 4 w0f loads and nothing more: the scalar sequencer processes DMA
  issues and activation dispatch through ONE FIFO, and a DMA issue that
  hits ring backpressure (reliably the 5th early issue) blocks every
  activation behind it -- measured ~12us whole-kernel regression from one
  extra 256KB scalar-ring DMA, and ~0.7us/tile PE stalls when outputs
  rode scalar.  The globally-last s-tile runs an asymmetric 384+128 split
  so the final act->stt->scan->mul(DVE)->dma chain covers 128 cols.
- Remaining fixed costs (measured, framework): ~6.3us head (boot + DMA
  ramp), ~5.8us NX dispatch tax (2.5ns x 2304 matmuls), ~8.8us TileContext
  teardown (2 all-engine barriers x 27 proc-clock waits at ~110ns each,
  count invariant to kernel structure).
"""

import numpy as np

import concourse.bacc as bacc
import concourse.mybir as mybir
from concourse import bass_utils
from concourse.tile import TileContext

# problem dims (hardcoded per spec)
B, S, D, H = 16, 2048, 512, 512
N_CORES = 8
BC = B // N_CORES  # batch rows per core
P = 128            # SBUF partitions
ST = 512           # s-tile (max moving free dim)
NS = S // ST       # 4 s-tiles
HC = H // P        # 4 hidden chunks
K0 = D // P        # 4 contraction chunks, layer 0
K1 = 2 * H // P    # 8 contraction chunks, layer 1

F32 = mybir.dt.float32
F16 = mybir.dt.float16
ACT = mybir.ActivationFunctionType
ALU = mybir.AluOpType


def build_nc():
    """Build the SPMD Bass program (same program on every core)."""
    nc = bacc.Bacc("TRN2", target_bir_lowering=False)

    # X host-prearranged p-major [BC, P, K0, S] so any (k, s) sub-block is a
    # single 3D-AP DMA; weights host-prearranged hh-major [P, HC, kc, 3*128]
    # (partition, hidden-chunk, contraction-chunk, gate*col) so ONE DMA per
    # hh unlocks every matmul of that hh (all k, all 3 gates) -- the head is
    # DMA-payload-bound and this makes first-needed bytes land first.
    xt = nc.dram_tensor("xt", [BC, P, K0, S], F16, kind="ExternalInput")
    w0f = nc.dram_tensor("w0f", [P, HC, K0, 3 * P], F16, kind="ExternalInput")
    w0b = nc.dram_tensor("w0b", [P, HC, K0, 3 * P], F16, kind="ExternalInput")
    w1f = nc.dram_tensor("w1f", [P, HC, K1, 3 * P], F16, kind="ExternalInput")
    w1b = nc.dram_tensor("w1b", [P, HC, K1, 3 * P], F16, kind="ExternalInput")
    # all 4 bias tables host-prearranged into ONE [P, 4, 12] tensor (slot
    # order bt0f, bt0b, bt1f, bt1b) -- a single plain 2D DMA
    ballh = nc.dram_tensor("ballh", [P, 4, 3 * HC], F32, kind="ExternalInput")
    # out rows indexed (dir*HC + hh)*P + p -> feature dir*H + hh*P + p
    out_t = nc.dram_tensor("out_t", [BC, 2 * HC, P, S], F16, kind="ExternalOutput")

    with TileContext(nc) as tc:
        with (
            tc.tile_pool(name="res", bufs=1) as rpool,
            tc.tile_pool(name="scr", bufs=4) as spool,
            tc.tile_pool(name="carry", bufs=2) as cpool,
            tc.tile_pool(name="ps", bufs=1, space="PSUM") as ppool,
        ):
            # ---- resident loads.  Weights are hh-major: one DMA per hh
            # covers all contraction chunks and gates of that hidden chunk.
            # Only w0f rides the scalar queue; the rest queue on sync,
            # transferred long before first use. ----
            def wload(wd, kc, name, eng):
                # one DMA for the whole weight tensor (contiguous per
                # partition) -- total DMA count is kept low on purpose
                wt = rpool.tile([P, HC, kc, 3 * P], F16, name=name)
                eng.dma_start(wt[:], wd[:])
                return wt

            ball = rpool.tile([P, 4, 3 * HC], F32, name="ball")

            xr = [rpool.tile([P, K0, S], F16, name=f"x{b}") for b in range(BC)]

            def xload(b, sl):
                nc.sync.dma_start(xr[b][:, :, sl], xt[b, :, :, sl])

            # Startup.  Measured facts driving this choreography:
            # - The graded exec window opens at ~5.8us no matter what (the
            #   framework emits gpsimd MEMSETs there), so early engine warmup
            #   work is FREE -- it cannot move first_useful.
            # - Operand payloads cannot land before ~12.2us: sequencer boot
            #   ~7.2, ~0.7us DIRECT2D issue per DMA, ring-FIFO payload
            #   drain shared by 16 HW engines.  First-needed bytes first:
            #   w0f (hh-major, hh0 unlocks 48 matmuls) on scalar, X s-slices
            #   on sync.
            # - The warmup activations read the dummy PSUM (forcing BOTH
            #   act-table loads early); their outputs go into `warm` (unused
            #   after the dummy matmuls; the WAR hazard orders them behind).
            wt0f = rpool.tile([P, HC, K0, 3 * P], F16, name="wt0f")
            # w0f in TWO half-loads: both issues fit inside scalar-ring
            # capacity (the 4th of 4 issues measurably stalls to ~11us on
            # backpressure, making whh3 visible only ~15.3us -- after its
            # first use).  Two issues at ~7.2/7.9us make hh2-3 visible
            # ~13.3us, before the PE needs them.
            nc.scalar.dma_start(wt0f[:, 0:2], w0f[:, 0:2])
            nc.scalar.dma_start(wt0f[:, 2:4], w0f[:, 2:4])
            warm = rpool.tile([P, ST], F16, name="warm")
            nc.gpsimd.memset(warm[:], 0.0)
            psw = ppool.tile([P, ST], F32, tag="ps0", name="psw", bufs=3)
            # 10 dummies fill PE-boot (~7.8us) to first-payload (~11-12.3us):
            # HAM flips to 2.4GHz at ~11 so real matmuls start warm.
            for _ in range(10):
                nc.tensor.matmul(psw[:], warm[:, 0:P], warm[:], start=True, stop=True)
            nc.scalar.activation(warm[:, 0:1], psw[:, 0:1], ACT.Tanh)
            nc.scalar.activation(warm[:, 1:2], psw[:, 1:2], ACT.Sigmoid)
            xload(0, slice(0, 256))
            nc.sync.dma_start(ball[:], ballh[:])
            bt0f, bt0b, bt1f, bt1b = 0, 1, 2, 3
            xload(0, slice(256, 512))
            xload(0, slice(512, 1024))
            xload(0, slice(1024, 2048))
            xload(1, slice(0, S))
            wt0b = wload(w0b, K0, "wt0b", nc.sync)
            wt1b = wload(w1b, K1, "wt1b", nc.sync)
            wt1f = wload(w1f, K1, "wt1f", nc.sync)

            # y1 resident, one tile per s-tile: [p, b, kk, s] with kk the
            # layer-1 contraction chunk (fw: 0..3, bw: 4..7)
            y1s = [rpool.tile([P, BC, K1, ST], F16, name=f"y1s{si}")
                   for si in range(NS)]

            def direction_pass(layer, fw, wt, bt, kc, last=False, first=False):
                dir_off = 0 if fw else HC
                s_order = range(NS) if fw else range(NS - 1, -1, -1)
                for b in range(BC):
                    carry = [cpool.tile([P, 1], F16, name=f"cr{b}_{hh}")
                             for hh in range(HC)]
                    # layer-1 outputs accumulate into full-row tiles and ship
                    # as ONE [128, 2048] DMA per (b, dir-hh) -- keeps total
                    # DMA count low (teardown probe) and halves issue work.
                    # The last pass's last row keeps per-s-tile writes so the
                    # final payload stays small (short tail).
                    if layer == 1 and not (last and b == BC - 1):
                        yrows = [spool.tile([P, S], F16, tag="yr",
                                            name=f"yr{hh}", bufs=4)
                                 for hh in range(HC)]
                    else:
                        yrows = None
                    for si, s_idx in enumerate(s_order):
                        s0 = s_idx * ST
                        # the kernel's globally-first s-tile runs h2-OUTER in
                        # 2x256 halves: all 4 hh consume only X[s0:256] (the
                        # xs0 DMA) for their first 48 matmuls, so the warm PE
                        # never outruns the xs1 payload (~14-17us) at the head
                        if first and b == 0 and si == 0:
                            SQ = ST // 2
                            tls = []
                            for hh in range(HC):
                                tls.append((
                                    [ppool.tile([P, ST], F32, tag=f"ps{g}",
                                                name=f"ps{g}",
                                                bufs=(3 if g < 2 else 2))
                                     for g in range(3)],
                                    spool.tile([P, ST], F16, tag="z", name="z"),
                                    spool.tile([P, ST], F16, tag="f", name="f"),
                                    spool.tile([P, ST], F16, tag="o", name="o"),
                                    spool.tile([P, ST], F16, tag="g", name="g"),
                                    spool.tile([P, ST], F16, tag="c", name="c"),
                                ))
                            for h2 in range(2):
                                cs = slice(h2 * SQ, (h2 + 1) * SQ)
                                for hh in range(HC):
                                    ps, z, f_, o, g_, c = tls[hh]
                                    bcol = lambda gi: ball[:, bt, gi * HC + hh : gi * HC + hh + 1]
                                    for g in range(3):
                                        for k in range(kc):
                                            nc.tensor.matmul(
                                                ps[g][:, cs],
                                                wt[:, hh, k, g * P:(g + 1) * P],
                                                xr[b][:, k, s0 + cs.start:s0 + cs.stop],
                                                start=(k == 0), stop=(k == kc - 1),
                                            )
                                    nc.scalar.activation(z[:, cs], ps[0][:, cs], ACT.Tanh, bias=bcol(0))
                                    nc.scalar.activation(f_[:, cs], ps[1][:, cs], ACT.Sigmoid, bias=bcol(1))
                                    nc.scalar.activation(o[:, cs], ps[2][:, cs], ACT.Sigmoid, bias=bcol(2))
                                    nc.vector.scalar_tensor_tensor(
                                        g_[:, cs], f_[:, cs], 1.0, z[:, cs], ALU.subtract, ALU.mult
                                    )
                                    init = 0.0 if h2 == 0 else c[:, cs.start - 1:cs.start]
                                    nc.vector.tensor_tensor_scan(
                                        c[:, cs], f_[:, cs], g_[:, cs], init,
                                        ALU.mult, ALU.subtract
                                    )
                                    nc.gpsimd.tensor_mul(
                                        y1s[s_idx][:, b, dir_off + hh, cs],
                                        o[:, cs], c[:, cs]
                                    )
                                    if h2 == 1:
                                        nc.gpsimd.tensor_copy(
                                            carry[hh][:], c[:, ST - 1:ST]
                                        )
                            continue
                        # the globally-last s-tile runs 2x256 halves so the
                        # final act->stt->scan->mul->dma chain after the last
                        # matmul is half as long
                        if last and b == BC - 1 and si == NS - 1:
                            # asymmetric 384+128 split: same matmul count as
                            # 256+256, but the post-last-matmul chain (acts ->
                            # stt -> scan -> mul -> dma) runs on 128 cols
                            bounds = [(0, 384), (384, ST)]
                            for hh in range(HC):
                                ps = [
                                    ppool.tile([P, ST], F32, tag=f"ps{g}",
                                               name=f"ps{g}", bufs=(3 if g < 2 else 2))
                                    for g in range(3)
                                ]
                                z = spool.tile([P, ST], F16, tag="z", name="z")
                                f_ = spool.tile([P, ST], F16, tag="f", name="f")
                                o = spool.tile([P, ST], F16, tag="o", name="o")
                                g_ = spool.tile([P, ST], F16, tag="g", name="g")
                                c = spool.tile([P, ST], F16, tag="c", name="c")
                                y = spool.tile([P, ST], F16, tag="y", name="y", bufs=2)
                                bcol = lambda gi: ball[:, bt, gi * HC + hh : gi * HC + hh + 1]
                                for h2, (c0, c1) in enumerate(bounds):
                                    cs = slice(c0, c1)
                                    for g in range(3):
                                        for k in range(kc):
                                            mov = (xr[b][:, k, s0 + cs.start:s0 + cs.stop]
                                                   if layer == 0
                                                   else y1s[s_idx][:, b, k, cs])
                                            nc.tensor.matmul(
                                                ps[g][:, cs], wt[:, hh, k, g * P:(g + 1) * P], mov,
                                                start=(k == 0), stop=(k == kc - 1),
                                            )
                                    nc.scalar.activation(z[:, cs], ps[0][:, cs], ACT.Tanh, bias=bcol(0))
                                    nc.scalar.activation(f_[:, cs], ps[1][:, cs], ACT.Sigmoid, bias=bcol(1))
                                    nc.scalar.activation(o[:, cs], ps[2][:, cs], ACT.Sigmoid, bias=bcol(2))
                                    nc.vector.scalar_tensor_tensor(
                                        g_[:, cs], f_[:, cs], 1.0, z[:, cs], ALU.subtract, ALU.mult
                                    )
                                    if h2 == 0:
                                        init = 0.0 if si == 0 else carry[hh][:]
                                    else:
                                        init = c[:, cs.start - 1:cs.start]
                                    nc.vector.tensor_tensor_scan(
                                        c[:, cs], f_[:, cs], g_[:, cs], init,
                                        ALU.mult, ALU.subtract
                                    )
                                    if layer == 0:
                                        nc.gpsimd.tensor_mul(
                                            y1s[s_idx][:, b, dir_off + hh, cs],
                                            o[:, cs], c[:, cs]
                                        )
                                    else:
                                        # DVE mul: at the tail the DVE is idle
                                        # right after the scan (same engine, no
                                        # cross-engine hop) and is ~2.5x faster
                                        # than the gpsimd mul
                                        nc.vector.tensor_mul(y[:, cs], o[:, cs], c[:, cs])
                                        nc.sync.dma_start(
                                            out_t[b, dir_off + hh, :, s0 + cs.start:s0 + cs.stop],
                                            y[:, cs]
                                        )
                                if si < NS - 1:
                                    nc.gpsimd.tensor_copy(
                                        carry[hh][:], c[:, ST - 1:ST]
                                    )
                            continue
                        for hh in range(HC):
                            ps = [
                                ppool.tile([P, ST], F32, tag=f"ps{g}",
                                           name=f"ps{g}", bufs=(3 if g < 2 else 2))
                                for g in range(3)
                            ]
                            for g in range(3):
                                for k in range(kc):
                                    mov = (xr[b][:, k, s0:s0 + ST] if layer == 0
                                           else y1s[s_idx][:, b, k, :])
                                    nc.tensor.matmul(
                                        ps[g][:], wt[:, hh, k, g * P:(g + 1) * P], mov,
                                        start=(k == 0), stop=(k == kc - 1),
                                    )
                            z = spool.tile([P, ST], F16, tag="z", name="z")
                            f_ = spool.tile([P, ST], F16, tag="f", name="f")
                            o = spool.tile([P, ST], F16, tag="o", name="o")
                            g_ = spool.tile([P, ST], F16, tag="g", name="g")
                            c = spool.tile([P, ST], F16, tag="c", name="c")
                            bcol = lambda gi: ball[:, bt, gi * HC + hh : gi * HC + hh + 1]
                            nc.scalar.activation(z[:], ps[0][:], ACT.Tanh, bias=bcol(0))
                            nc.scalar.activation(f_[:], ps[1][:], ACT.Sigmoid, bias=bcol(1))
                            nc.scalar.activation(o[:], ps[2][:], ACT.Sigmoid, bias=bcol(2))
                            # g' = (f-1)*z; scan: c = f*c - g' = f*c + (1-f)*z
                            nc.vector.scalar_tensor_tensor(
                                g_[:], f_[:], 1.0, z[:], ALU.subtract, ALU.mult
                            )
                            if fw:
                                sc = (c[:], f_[:], g_[:])
                                ccol = slice(ST - 1, ST)
                            else:
                                sc = (c[:, ::-1], f_[:, ::-1], g_[:, ::-1])
                                ccol = slice(0, 1)
                            init = 0.0 if si == 0 else carry[hh][:]
                            nc.vector.tensor_tensor_scan(
                                sc[0], sc[1], sc[2], init, ALU.mult, ALU.subtract
                            )
                            if si < NS - 1:
                                nc.gpsimd.tensor_copy(carry[hh][:], c[:, ccol])
                            if layer == 0:
                                nc.gpsimd.tensor_mul(
                                    y1s[s_idx][:, b, dir_off + hh, :], o[:], c[:]
                                )
                            elif yrows is not None:
                                nc.gpsimd.tensor_mul(
                                    yrows[hh][:, s0:s0 + ST], o[:], c[:]
                                )
                                if si == NS - 1:
                                    # all 4 s-tiles of this (b, dir-hh) done
                                    nc.sync.dma_start(
                                        out_t[b, dir_off + hh, :, :],
                                        yrows[hh][:]
                                    )
                            else:
                                y = spool.tile([P, ST], F16, tag="y", name="y", bufs=2)
                                nc.gpsimd.tensor_mul(y[:], o[:], c[:])
                                # out-writes on the sync ring only: a DMA
                                # issue costs ~0.6us of SEQUENCER time, and a
                                # DIRECT2D on the scalar sequencer blocks the
                                # act stream behind it (measured ~0.7us/tile
                                # PE stalls in l1-bw si2).
                                nc.sync.dma_start(
                                    out_t[b, dir_off + hh, :, s0:s0 + ST], y[:]
                                )

            direction_pass(0, True, wt0f, bt0f, K0, first=True)
            direction_pass(0, False, wt0b, bt0b, K0)
            direction_pass(1, False, wt1b, bt1b, K1)
            direction_pass(1, True, wt1f, bt1f, K1, last=True)

    nc.finalize()
    return nc


_NC_CACHE = {}


def _get_nc(variant="fp16"):
    if variant not in _NC_CACHE:
        _NC_CACHE[variant] = build_nc()
    return _NC_CACHE[variant]


def kernel(X, seqlens, W_fw0, b_fw0, W_bw0, b_bw0, W_fw1, b_fw1, W_bw1, b_bw1,
           mm_dtype="fp16", trace=False):
    """Full-input entry point: shards over 8 cores, returns [B, S, 2H] f32."""
    del seqlens  # unused by the reference computation
    X = np.asarray(X, dtype=np.float32)

    def wprep(w, kc):  # [Din, 3H] f32 -> hh-major [P, HC, kc, 3*128] fp16
        w5 = np.asarray(w, np.float32).reshape(kc, P, 3, HC, P)
        return np.ascontiguousarray(
            w5.transpose(1, 3, 0, 2, 4).reshape(P, HC, kc, 3 * P)
            .astype(np.float16)
        )

    def bprep(b):  # [3H] f32 -> [P, 12] f32 (partition-major bias table)
        return np.ascontiguousarray(
            np.asarray(b, np.float32).reshape(3 * HC, P).T
        )

    weights = {
        "w0f": wprep(W_fw0, K0), "w0b": wprep(W_bw0, K0),
        "w1f": wprep(W_fw1, K1), "w1b": wprep(W_bw1, K1),
        # slot order bt0f, bt0b, bt1f, bt1b
        "ballh": np.ascontiguousarray(np.stack(
            [bprep(b_fw0), bprep(b_bw0), bprep(b_fw1), bprep(b_bw1)], axis=1)),
    }

    nc = _get_nc(mm_dtype)
    in_maps = []
    for i in range(N_CORES):
        rows = X[i * BC : (i + 1) * BC]  # [BC, S, D]
        xt_i = np.ascontiguousarray(
            rows.transpose(0, 2, 1).reshape(BC, K0, P, S)
            .transpose(0, 2, 1, 3).astype(np.float16)  # p-major [BC,P,K0,S]
        )
        in_maps.append({"xt": xt_i, **weights})

    res = bass_utils.run_bass_kernel_spmd(
        nc, in_maps, core_ids=list(range(N_CORES)), trace=trace
    )
    out = np.empty((B, S, 2 * H), dtype=np.float32)
    for i in range(N_CORES):
        out_t = res.results[i]["out_t"]  # [BC, 2*HC, P, S] fp16
        out[i * BC : (i + 1) * BC] = (
            out_t.reshape(BC, 2 * H, S).transpose(0, 2, 1).astype(np.float32)
        )
    kernel.last_results = res
    return out

